# revision 2
# baseline (speedup 1.0000x reference)
"""GST-LSTM cell (graph-conv LSTM) on 8 Trainium2 NeuronCores.

Computation (reference):
    g  = adj_matrix @ Ht_1                       # (N, H)  -- dominant cost
    i  = sigmoid(ht @ Wxi.T + bxi + g @ Whi.T + bhi)
    f  = sigmoid(ht @ Wxf.T + bxf + g @ Whf.T + bhf)
    o  = sigmoid(ht @ Wxo.T + bxo + g @ Who.T + bho)
    u  = tanh   (ht @ Wxc.T + bxc + g @ Whc.T + bhc)
    Ct = f * Ct_1 + i * u
    Ht = o * tanh(Ct)

Sharding: node dim (rows of adj, ht, Ct_1; output rows) split across the
8 cores; Ht_1 replicated. No collectives needed.

Device layout: everything is computed feature-major ([64, nodes] tiles)
so that the PE contraction dim lands on partitions without any on-device
transposes:
  - the adjacency slice is host-transposed, shifted by -0.5, cast to
    fp16, and PRE-LINEARIZED so that each device stripe ([128, ktb*512])
    is one fully contiguous DRAM block (128 lines of ktb KiB back to
    back. The DMA engines then see pure sequential HBM reads, which is
    what the memory-bound regime rewards.
  - hh packs [H_hi | H_lo] fp16 side by side per k-tile: one [128,128]
    stationary computes both products in a single matmul pass (PSUM rows
    0:64 accumulate A@H_hi, rows 64:128 accumulate A@H_lo), so the fp16
    quantization of Ht_1 is corrected for free on the PE.
  - the eight 64x64 Linears run in fp32 (weights host-pre-transposed),
    also feature-major; biases enter via the ACT engine's per-partition
    bias operand together with the sigmoid/tanh.
  - gating is elementwise on DVE/ACT; outputs leave as [64, 2048] fp32
    and the host transposes them back.

Adjacency stripes are issued on the SP HWDGE queue; output DMAs go on
the Activation HWDGE queue so they cannot head-of-line block the
adjacency stream on SP's in-order sequencer.

adj is shifted by -0.5 before the fp16 cast (halves quantization error
for uniform(0,1) entries). g = (adj-0.5)@H + 0.5*colsum(H) broadcast
over rows; the colsum term passes through the h-side Linear as a
per-feature constant, folded into the gate bias on the host in float64.
"""

import numpy as np

N = 16384
D = 64
N_CORES = 8
ROWS = N // N_CORES          # 2048 nodes per core
MBW = 512                    # m-block width (PE moving free dim / PSUM bank)
MB = ROWS // MBW             # 4 m-blocks per core
KT = N // 128                # 128 k-tiles of 128 contraction rows
KTB = 8                      # k-tiles per stripe DMA (1 MiB contiguous)
GD = KT // KTB               # stripe DMAs per m-block


def _split_excess_waits(nc, max_waits=1):
    """Split >max_waits sem waits off instructions onto preceding NOPs.

    The walrus build here rejects instructions carrying more than a
    couple of sync waits ("Too many sync wait commands" from
    setupSyncWait during codegen). Tile's wait assignment doesn't know
    that limit; an NX engine executes its stream in order, so moving
    the excess waits onto same-engine NOPs directly before the
    instruction preserves ordering semantics with a legal encoding.
    """
    from concourse import mybir

    fn = nc.m.functions[0]
    for bb in fn.blocks:
        out = []
        for inst in bb.instructions:
            si = getattr(inst, "sync_info", None)
            if si is not None and si.on_wait and len(si.on_wait) > max_waits:
                waits = list(si.on_wait)
                spill, keep = waits[:-max_waits], waits[-max_waits:]
                for i in range(0, len(spill), max_waits):
                    nop = mybir.InstNoOp(
                        name=nc.get_next_instruction_name(),
                        sync_info=mybir.SyncInfo(
                            on_wait=spill[i:i + max_waits], on_update=[]
                        ),
                        bass_nofuse=True,
                        engine=inst.engine,
                    )
                    out.append(nop)
                si.on_wait = keep
            out.append(inst)
        bb.instructions[:] = out


_GATE_FUNCS = ("Sigmoid", "Sigmoid", "Sigmoid", "Tanh")  # i, f, o, u


def build(n=N, rows=ROWS, mbw=MBW, ktb=KTB, repeat=1, adj_bufs=6,
          split_waits=True, out_engine="scalar"):
    """Build the per-core Bass program. All cores run the same program."""
    import concourse.bass as bass
    import concourse.mybir as mybir
    from concourse import tile

    dt = mybir.dt
    act = mybir.ActivationFunctionType
    f16, f32 = dt.float16, dt.float32

    kt = n // 128
    mb = rows // mbw
    gd = kt // ktb

    nc = bass.Bass()
    # host-linearized adjacency: stripe (mbi, g) is the contiguous block
    # adjl[(mbi*gd + g)*128 : ...+128, :]
    adjl = nc.declare_dram_parameter(
        "adjl", [mb * gd * 128, ktb * mbw], f16, isOutput=False
    )
    hh = nc.declare_dram_parameter("hh", [128, kt * 2 * D], f16, isOutput=False)
    xt = nc.declare_dram_parameter("xt", [D, rows], f32, isOutput=False)
    ct = nc.declare_dram_parameter("ct", [D, rows], f32, isOutput=False)
    wxt = nc.declare_dram_parameter("wxt", [D, 4 * D], f32, isOutput=False)
    wht = nc.declare_dram_parameter("wht", [D, 4 * D], f32, isOutput=False)
    bias = nc.declare_dram_parameter("bias", [D, 4], f32, isOutput=False)
    ht_out = nc.declare_dram_parameter("ht_out", [D, rows], f32, isOutput=True)
    ct_out = nc.declare_dram_parameter("ct_out", [D, rows], f32, isOutput=True)

    adjl_r = adjl[:].rearrange("(mb g p) x -> mb g p x", g=gd, p=128)
    out_eng = getattr(nc, out_engine)

    with tile.TileContext(nc) as tc:
        with (
            tc.tile_pool(name="const", bufs=1) as cst,
            tc.tile_pool(name="adj", bufs=adj_bufs) as apool,
            tc.tile_pool(name="b64", bufs=3) as b64,
            tc.tile_pool(name="gpsum", bufs=2, space="PSUM") as gpsum,
            tc.tile_pool(name="gatepsum", bufs=3, space="PSUM") as gatepsum,
        ):
            hh_sb = cst.tile([128, kt * 2 * D], f16)
            nc.sync.dma_start(hh_sb[:], hh[:])
            wxt_sb = cst.tile([D, 4 * D], f32)
            nc.sync.dma_start(wxt_sb[:], wxt[:])
            wht_sb = cst.tile([D, 4 * D], f32)
            nc.sync.dma_start(wht_sb[:], wht[:])
            bias_sb = cst.tile([D, 4], f32)
            nc.sync.dma_start(bias_sb[:], bias[:])
            xt_sb = cst.tile([D, rows], f32)
            nc.sync.dma_start(xt_sb[:], xt[:])
            ct_sb = cst.tile([D, rows], f32)
            nc.sync.dma_start(ct_sb[:], ct[:])

            def body(_iv=None):
                for mbi in range(mb):
                    mbs = slice(mbi * mbw, (mbi + 1) * mbw)
                    gps = gpsum.tile([128, mbw], f32, tag="gps")
                    for g in range(gd):
                        stripe = apool.tile([128, ktb * mbw], f16, tag="stripe")
                        nc.sync.dma_start(stripe[:], adjl_r[mbi, g])
                        for a in range(ktb):
                            kti = g * ktb + a
                            nc.tensor.matmul(
                                gps[:],
                                hh_sb[:, kti * 2 * D:(kti + 1) * 2 * D],
                                stripe[:, a * mbw:(a + 1) * mbw],
                                start=(kti == 0),
                                stop=(kti == kt - 1),
                            )
                    gtb = b64.tile([D, mbw], f32, tag="gtb")
                    nc.vector.tensor_copy(gtb[:], gps[0:D, :])
                    nc.vector.tensor_add(gtb[:], gtb[:], gps[D:2 * D, :])

                    gates = []
                    for gi, fname in enumerate(_GATE_FUNCS):
                        pg = gatepsum.tile([D, mbw], f32, tag="pg")
                        nc.tensor.matmul(
                            pg[:],
                            wxt_sb[:, gi * D:(gi + 1) * D],
                            xt_sb[:, mbs],
                            start=True,
                            stop=False,
                        )
                        nc.tensor.matmul(
                            pg[:],
                            wht_sb[:, gi * D:(gi + 1) * D],
                            gtb[:],
                            start=False,
                            stop=True,
                        )
                        gate_sb = b64.tile([D, mbw], f32, tag=f"gate{gi}")
                        nc.scalar.activation(
                            gate_sb[:],
                            pg[:],
                            getattr(act, fname),
                            bias=bias_sb[:, gi:gi + 1],
                        )
                        gates.append(gate_sb)
                    it_, ft_, ot_, ut_ = gates

                    t1 = b64.tile([D, mbw], f32, tag="t1")
                    nc.vector.tensor_mul(t1[:], ft_[:], ct_sb[:, mbs])
                    t2 = b64.tile([D, mbw], f32, tag="t2")
                    nc.vector.tensor_mul(t2[:], it_[:], ut_[:])
                    ctn = b64.tile([D, mbw], f32, tag="ctn")
                    nc.vector.tensor_add(ctn[:], t1[:], t2[:])
                    out_eng.dma_start(ct_out[:, mbs], ctn[:])
                    tct = b64.tile([D, mbw], f32, tag="tct")
                    nc.scalar.activation(tct[:], ctn[:], act.Tanh)
                    htn = b64.tile([D, mbw], f32, tag="htn")
                    nc.vector.tensor_mul(htn[:], ot_[:], tct[:])
                    out_eng.dma_start(ht_out[:, mbs], htn[:])

            if repeat == 1:
                body()
            else:
                with tc.For_i(0, repeat, 1) as _i:
                    body(_i)

    if split_waits:
        _split_excess_waits(nc)
    return nc


def make_in_maps(inputs, n=N, n_cores=N_CORES, mbw=MBW, ktb=KTB):
    """Host-side sharding + relayout. Returns per-core input dicts."""
    rows = n // n_cores
    kt = n // 128
    mb = rows // mbw
    gd = kt // ktb
    adj = np.asarray(inputs["adj_matrix"], dtype=np.float32)
    H = np.asarray(inputs["Ht_1"], dtype=np.float32)
    ht = np.asarray(inputs["ht"], dtype=np.float32)
    Ct_1 = np.asarray(inputs["Ct_1"], dtype=np.float32)

    # H = hi + lo to ~2^-22: the hi/lo fp16 pair is packed side by side
    # per k-tile ([128, kt*128]) so one matmul computes both products.
    Hh32 = H.astype(np.float16).astype(np.float32)
    packed = np.empty((128, kt, 2 * D), dtype=np.float16)
    packed[:, :, :D] = Hh32.reshape(kt, 128, D).transpose(1, 0, 2)
    packed[:, :, D:] = (H - Hh32).reshape(kt, 128, D).transpose(1, 0, 2)
    hh = np.ascontiguousarray(packed.reshape(128, kt * 2 * D))

    gate_w = ("Wxi", "Wxf", "Wxo", "Wxc")
    gate_h = ("Whi", "Whf", "Who", "Whc")
    wxt = np.concatenate(
        [np.asarray(inputs[g + "_w"], np.float32).T for g in gate_w], axis=1
    )
    wht = np.concatenate(
        [np.asarray(inputs[g + "_w"], np.float32).T for g in gate_h], axis=1
    )
    colsum = H.astype(np.float64).sum(axis=0)
    bias = np.stack(
        [
            np.asarray(inputs[gx + "_b"], np.float64)
            + np.asarray(inputs[gh + "_b"], np.float64)
            + 0.5 * (np.asarray(inputs[gh + "_w"], np.float64) @ colsum)
            for gx, gh in zip(gate_w, gate_h)
        ],
        axis=1,
    ).astype(np.float32)
    wxt = np.ascontiguousarray(wxt)
    wht = np.ascontiguousarray(wht)
    bias = np.ascontiguousarray(bias)

    in_maps = []
    for c in range(n_cores):
        rs = slice(c * rows, (c + 1) * rows)
        adjt_c = np.ascontiguousarray(adj[rs].T)
        adjt_c -= np.float32(0.5)
        a16 = adjt_c.astype(np.float16)
        # linearize: stripe (mbi, g) contiguous, lines of ktb*mbw fp16
        adjl = np.ascontiguousarray(
            a16.reshape(gd, ktb, 128, mb, mbw)
            .transpose(3, 0, 2, 1, 4)
            .reshape(mb * gd * 128, ktb * mbw)
        )
        in_maps.append(
            {
                "adjl": adjl,
                "hh": hh,
                "xt": np.ascontiguousarray(ht[rs].T),
                "ct": np.ascontiguousarray(Ct_1[rs].T),
                "wxt": wxt,
                "wht": wht,
                "bias": bias,
            }
        )
    return in_maps


def gather(results):
    Ht = np.concatenate([r["ht_out"].T for r in results], axis=0)
    Ct = np.concatenate([r["ct_out"].T for r in results], axis=0)
    return np.ascontiguousarray(Ht), np.ascontiguousarray(Ct)


_PROGRAM_CACHE = {}


def kernel(**inputs):
    from concourse.bass_utils import run_bass_kernel_spmd

    if "nc" not in _PROGRAM_CACHE:
        _PROGRAM_CACHE["nc"] = build()
    nc = _PROGRAM_CACHE["nc"]
    in_maps = make_in_maps(inputs)
    res = run_bass_kernel_spmd(nc, in_maps, list(range(N_CORES)))
    return gather(res.results)
